# revision 10
# baseline (speedup 1.0000x reference)
"""Trainium2 Bass kernel: contrastive loss with negative mining.

Math:
    centers  = mean over contiguous chunks of 8 rows               [n/8, d]
    x_pos    = x + 0.5*(center - x)        => |x - x_pos| = 0.5*|x - center|
    sim      = x @ x.T                                             [n, n]
    neg_idx  = argmax_j sim[i, j] excluding j in i's group-of-4
    d_an     = mean_d |x - x_neg|,  d_ap = mean_d |x - x_pos|
    loss     = sum( (1/8) * d_ap / (d_an + 1e-7) )

Distribution: sim is symmetric, so only the upper block-triangle is
computed (plus a uniform sliver of duplicates).  Skew decomposition keeps
the program identical on all 8 cores (SPMD): core c owns block-rows
a = 8k + c (k = slot 0..7); slot k is matmul'd against column strips
[2k, 15].  Every sim element (r, j) is then covered either by r's own
row side (j-block >= 8k) or by the transpose of block (j-block, r-block)
computed on j-block's owner.

Per (slot, strip) task: fp8 DoubleRow matmuls -> PSUM f32 -> bf16 SBUF
strip.  Row side: per-strip reduce-max into RC, strip data staged to DRAM;
after a slot's sweep the winning strip is re-fetched by indirect DMA and
FIND_INDEX8 recovers the column.  Col side: PE transposes (strips >=
2k+2) land in per-strip windows; reduce-max + FIND_INDEX8 give per-column
(value, source-row) candidates.  Candidates are packed into f32 as
floor(4*v)*8192 + j (integer-exact, argmax-compatible; quantization
mispicks are benign) and ReduceScatter(max) routes each row's best
candidate to its owner.  Two RS rounds (strips 15..8, then 7..0) overlap
the first half's mining tail with the second half's matmuls.  The owner
unpacks j = w mod 8192, gathers x_neg, and computes exact d_an/d_ap/loss.
"""

import math

import ml_dtypes
import numpy as np

import concourse.bass as bass
import concourse.mybir as mybir
import concourse.tile as tile
from concourse import bacc
from concourse.bass import IndirectOffsetOnAxis
from concourse.bass_utils import run_bass_kernel_spmd

BF16 = mybir.dt.bfloat16
F32 = mybir.dt.float32
U32 = mybir.dt.uint32
ALU = mybir.AluOpType
ACTF = mybir.ActivationFunctionType
AXX = mybir.AxisListType.X

P = 128          # partitions / block height
JS = 512         # strip width (4 blocks)
CHUNK = 8
GROUP = 4
WEIGHT = 1.0 / 8
EPS = 1e-7
NEG_BIG = -1e30          # mask addend / empty-candidate fill
PACK_NEG = -3.0e34       # pre-packed equivalent of NEG_BIG (loses to all real)
RK = 12582912.0          # 1.5*2^23: adding+subtracting rounds f32 to integer


class Cfg:
    def __init__(self, n=8192, d=2048, cores=8, fp8=True):
        self.n, self.d, self.cores, self.fp8 = n, d, cores, fp8
        self.r = n // cores           # rows per core (1024)
        self.slots = self.r // P      # stationary slots per core (8)
        self.ns = n // JS             # column strips (16)
        self.kb = d // P              # contraction blocks (16)
        self.cw = min(d, JS)          # d-chunk width for d_ap matmul
        self.ch = d // self.cw
        assert n == 8192 and d == 2048 and cores == 8


def _body(tc: tile.TileContext, cfg: Cfg, io: dict):
    nc = tc.nc
    pools = {}

    def pool(name, bufs, space="SBUF"):
        if name not in pools:
            pools[name] = tc.alloc_tile_pool(name=name, bufs=bufs, space=space)
        return pools[name]

    n, d, ns, kb = cfg.n, cfg.d, cfg.ns, cfg.kb
    S = cfg.slots   # 8

    # ---------------- resident inputs ----------------
    xs_sb = pool("xs", 1).tile([P, kb * cfg.r], mybir.dt.float8e4, name="xs_sb")
    for k in range(0, kb, 2):
        ke = min(k + 2, kb)
        nc.sync.dma_start(
            out=xs_sb[:, k * cfg.r:ke * cfg.r].rearrange(
                "p (a r) -> p a r", a=ke - k),
            in_=io["xs"][k * P:ke * P, :].rearrange("(a p) r -> p a r", p=P),
        )
    xs3 = xs_sb[:].rearrange("p (a r) -> p a r", a=kb)

    consts = pool("consts", 1)
    eye_sb = consts.tile_from(io["eye"])         # [128,128] bf16 identity
    m2b_sb = consts.tile_from(io["m2b"])         # [128,128] bf16 d_ap matrix
    maskp_sb = consts.tile_from(io["maskp"])     # [128,1024] bf16 diag mask
    iota16_sb = consts.tile_from(io["iota16"])   # [128,16] f32 0..15
    bmi16_sb = consts.tile_from(io["bmi16"])     # [128,16] f32 BIG - iota
    p16_sb = consts.tile_from(io["p16"])         # [128,1] f32 p*16
    c128_sb = consts.tile_from(io["c128"])       # [128,1] f32 128*c
    negfill_sb = consts.tile_from(io["negfill"])  # [128,32] f32 PACK_NEG

    # xrb (bf16 rows of this core) loads late-ish; needed for d_an/d_ap only
    xrb_sb = pool("xrb", 1).tile([P, S * d], BF16, name="xrb_sb")

    # ---------------- DRAM scratch ----------------
    dram = pool("dram", 1, space="DRAM")
    rowstrips = dram.tile([S * P * ns, JS], BF16, name="rowstrips")
    bounce_hi = dram.tile([1, 4096], F32, name="bounce_hi")
    bounce_hi_out = dram.tile([1, 512], F32, name="bounce_hi_out")
    bounce_lo = dram.tile([1, 4096], F32, name="bounce_lo")
    bounce_lo_out = dram.tile([1, 512], F32, name="bounce_lo_out")

    # memset bounce_lo (b<8 slots never written)
    nc.sync.dma_start(
        out=bounce_lo[:].rearrange("o (q f) -> (o q) f", q=P),
        in_=negfill_sb[:],
    )

    # ---------------- persistent SBUF state ----------------
    state = pool("state", 1)
    RC = state.tile([P, S * ns], F32, name="RC")          # row chunk maxes
    nc.gpsimd.memset(RC[:], NEG_BIG)
    colm_hi = state.tile([P, 32], F32, name="colm_hi")    # col max, b-32
    colp_hi = state.tile([P, 32], F32, name="colp_hi")    # col win pos (f32)
    colm_lo = state.tile([P, 32], F32, name="colm_lo")    # col max, b-... b in 8..31 at b-0? use b index-8? see below
    colp_lo = state.tile([P, 32], F32, name="colp_lo")
    nc.gpsimd.memset(colm_lo[:], NEG_BIG)
    nc.gpsimd.memset(colp_lo[:], 0.0)
    rowm = state.tile([P, S], F32, name="rowm")           # row max per slot
    jrow = state.tile([P, S], F32, name="jrow")           # row argmax col
    wfin = state.tile([P, S], F32, name="wfin")           # final packed
    san = state.tile([P, S], F32, name="san")             # sum |x - xneg|
    sap = state.tile([P, S * cfg.ch], F32, name="sap")    # sum |y| per chunk

    xmp = pool("xm", 2)
    psum = pool("ps", 6, space="PSUM")
    pt_pool = pool("pt", 2, space="PSUM")
    evac = pool("evac", 3)
    winp = pool("win", 2)
    small = pool("small", 2)
    gath = pool("gath", 2)
    xneg_p = pool("xneg", 2)
    diff_p = pool("diff", 2)
    yabs = pool("yabs", 2)

    def matmul_task(k, s, out_ps):
        """fp8 DoubleRow sim matmuls: slot k stationary x strip s."""
        for q in range(0, kb, 2):
            nc.tensor.matmul(
                out=out_ps[:],
                lhsT=xs3[:, q:q + 2, k * P:(k + 1) * P],
                rhs=xm3[:, q:q + 2, :],
                start=(q == 0), stop=(q == kb - 2),
                perf_mode=mybir.MatmulPerfMode.DoubleRow,
            )

    def row_extract(k):
        """After slot k's strips all reduced: find its row-side winner."""
        sm = small.tile([P, 16], F32, name=f"sm{k}", tag="sm")
        # max over strips
        nc.vector.tensor_reduce(
            out=rowm[:, k:k + 1], in_=RC[:, k * ns:(k + 1) * ns].rearrange(
                "p (o f) -> p o f", o=1), axis=AXX, op=ALU.max)
        # strip argmax (lowest strip on ties)
        oh = small.tile([P, 16], F32, name=f"oh{k}", tag="oh")
        nc.vector.tensor_tensor(
            out=oh[:], in0=RC[:, k * ns:(k + 1) * ns],
            in1=rowm[:, k:k + 1].to_broadcast([P, 16]), op=ALU.is_ge)
        nc.vector.tensor_tensor(
            out=sm[:], in0=oh[:], in1=bmi16_sb[:], op=ALU.mult)
        sel = small.tile([P, 1], F32, name=f"sel{k}", tag="sel")
        # sel = BIG - max(oh*(BIG-iota)) = min strip index achieving max
        nc.vector.tensor_reduce(
            out=sel[:], in_=sm[:].rearrange("p (o f) -> p o f", o=1),
            axis=AXX, op=ALU.max)
        nc.vector.tensor_scalar(
            out=sel[:], in0=sel[:], scalar1=-1.0, scalar2=65536.0,
            op0=ALU.mult, op1=ALU.add)
        # gather winning strip from DRAM: row id = p*16 + sel
        offs = small.tile([P, 1], F32, name=f"offs{k}", tag="offs")
        nc.vector.scalar_tensor_tensor(
            out=offs[:], in0=sel[:], scalar=float(k * P * ns), in1=p16_sb[:],
            op0=ALU.add, op1=ALU.add)
        offu = small.tile([P, 1], U32, name=f"offu{k}", tag="offu")
        nc.vector.tensor_copy(out=offu[:], in_=offs[:])
        gstrip = gath.tile([P, JS], BF16, name="gstrip")
        nc.gpsimd.indirect_dma_start(
            out=gstrip[:], out_offset=None,
            in_=rowstrips[:, :],
            in_offset=IndirectOffsetOnAxis(ap=offu[:], axis=0),
            bounds_check=S * P * ns - 1, oob_is_err=False,
        )
        mkb = small.tile([P, 8], BF16, name=f"mkb{k}", tag="mkb")
        nc.vector.tensor_copy(out=mkb[:], in_=rowm[:, k:k + 1].to_broadcast([P, 8]))
        pos8 = small.tile([P, 8], U32, name=f"pos8{k}", tag="pos8")
        nc.vector.max_index(out=pos8[:], in_max=mkb[:], in_values=gstrip[:])
        posf = small.tile([P, 1], F32, name=f"posf{k}", tag="posf")
        nc.vector.tensor_copy(out=posf[:], in_=pos8[:, 0:1])
        # j = sel*512 + pos
        nc.vector.scalar_tensor_tensor(
            out=jrow[:, k:k + 1], in0=sel[:], scalar=float(JS), in1=posf[:],
            op0=ALU.mult, op1=ALU.add)

    def pack_half(colm, colp, rowk0, bounce, nslots):
        """Pack col (and row) candidates to w = floor(4v)*8192 + j; write col
        packed to the bounce buffer."""
        # col side: j = 8*p - 7*(p mod 128) + 128*c, all via the
        # RK round-trick (DVE ISA has no mod).  kf = floor(p/128) exactly:
        # round((p+0.5)/128 - 0.5 + RK) - RK.
        kf = small.tile([P, 32], F32, name="kf", tag="pk_kf")
        nc.vector.tensor_scalar(
            out=kf[:], in0=colp[:], scalar1=0.5, scalar2=1.0 / 128,
            op0=ALU.add, op1=ALU.mult)
        nc.vector.tensor_scalar(
            out=kf[:], in0=kf[:], scalar1=-0.5, scalar2=RK,
            op0=ALU.add, op1=ALU.add)
        nc.vector.tensor_scalar(
            out=kf[:], in0=kf[:], scalar1=RK, scalar2=None, op0=ALU.subtract)
        pm = small.tile([P, 32], F32, name="pm", tag="pk_pm")
        nc.vector.scalar_tensor_tensor(
            out=pm[:], in0=kf[:], scalar=-128.0, in1=colp[:],
            op0=ALU.mult, op1=ALU.add)
        t2 = small.tile([P, 32], F32, name="t2", tag="pk_t2")
        nc.vector.tensor_scalar(
            out=t2[:], in0=pm[:], scalar1=-7.0, scalar2=None, op0=ALU.mult)
        jc = small.tile([P, 32], F32, name="jc", tag="pk_jc")
        nc.vector.scalar_tensor_tensor(
            out=jc[:], in0=colp[:], scalar=8.0, in1=t2[:],
            op0=ALU.mult, op1=ALU.add)
        nc.vector.tensor_tensor(
            out=jc[:], in0=jc[:], in1=c128_sb[:].to_broadcast([P, 32]),
            op=ALU.add)
        # value quantized to 0.5: rq = round(2v); w = rq*8192 + j + 0.5
        rq = small.tile([P, 32], F32, name="rq", tag="pk_rq")
        nc.vector.tensor_scalar(
            out=rq[:], in0=colm[:], scalar1=2.0, scalar2=RK,
            op0=ALU.mult, op1=ALU.add)
        nc.vector.tensor_scalar(
            out=rq[:], in0=rq[:], scalar1=RK, scalar2=None, op0=ALU.subtract)
        wc = small.tile([P, 32], F32, name="wc", tag="pk_wc")
        nc.vector.scalar_tensor_tensor(
            out=wc[:], in0=rq[:], scalar=8192.0, in1=jc[:],
            op0=ALU.mult, op1=ALU.add)
        nc.vector.tensor_scalar(
            out=wc[:], in0=wc[:], scalar1=0.5, scalar2=None, op0=ALU.add)
        # scatter to bounce: element (p, i) -> slot cb*1024... within-half:
        # half-local b index i (0..31), global kb half handled by caller.
        # bounce flat index = cb*512 + kbl*128 + p, where i = kbl*8 + ... no:
        # i = b - b0 with b natural order => i = kbl*... b = b0 + i,
        # cb = b mod 8, kbl = (b - b0)//8 = i//8, and i mod 8 = cb' pattern:
        # b natural ascending => i = kbl*8 + cb? b = b0 + kbl*8 + cb yes.
        nc.sync.dma_start(
            out=bounce[:].rearrange("o (cb kbl q) -> (o q) cb kbl",
                                    cb=8, kbl=4),
            in_=wc[:].rearrange("p (cb kbl) -> p cb kbl", cb=8),
        )

    def finish_slots(klo, khi, bounce_out):
        """RS result -> final winner -> gather x_neg -> d_an for slots
        [klo, khi)."""
        nsl = khi - klo
        colw = small.tile([P, nsl], F32, name=f"colw{klo}", tag="cw_colw")
        nc.sync.dma_start(
            out=colw[:],
            in_=bounce_out[:].rearrange("o (kk q) -> (o q) kk", q=P),
        )
        # pack row-side candidates for these slots (same format)
        rqr = small.tile([P, nsl], F32, name=f"rqr{klo}", tag="cw_rqr")
        nc.vector.tensor_scalar(
            out=rqr[:], in0=rowm[:, klo:khi], scalar1=2.0, scalar2=RK,
            op0=ALU.mult, op1=ALU.add)
        nc.vector.tensor_scalar(
            out=rqr[:], in0=rqr[:], scalar1=RK, scalar2=None,
            op0=ALU.subtract)
        wr = small.tile([P, nsl], F32, name=f"wr{klo}", tag="cw_wr")
        nc.vector.scalar_tensor_tensor(
            out=wr[:], in0=rqr[:], scalar=8192.0, in1=jrow[:, klo:khi],
            op0=ALU.mult, op1=ALU.add)
        nc.vector.tensor_scalar(
            out=wr[:], in0=wr[:], scalar1=0.5, scalar2=None, op0=ALU.add)
        nc.vector.tensor_tensor(
            out=wfin[:, klo:khi], in0=wr[:], in1=colw[:], op=ALU.max)
        # unpack j = w mod 8192: m = floor(w/8192) via round(w/8192 - 0.5),
        # exact since frac(w/8192) = (j+0.5)/8192 is never 0 or 1/2... then
        # j = w - 8192*m - 0.5.
        mf = small.tile([P, nsl], F32, name=f"mf{klo}", tag="cw_mf")
        nc.vector.tensor_scalar(
            out=mf[:], in0=wfin[:, klo:khi], scalar1=1.0 / 8192,
            scalar2=-0.5, op0=ALU.mult, op1=ALU.add)
        nc.vector.tensor_scalar(
            out=mf[:], in0=mf[:], scalar1=RK, scalar2=None, op0=ALU.add)
        nc.vector.tensor_scalar(
            out=mf[:], in0=mf[:], scalar1=RK, scalar2=None, op0=ALU.subtract)
        jf = small.tile([P, nsl], F32, name=f"jf{klo}", tag="cw_jf")
        nc.vector.scalar_tensor_tensor(
            out=jf[:], in0=mf[:], scalar=-8192.0, in1=wfin[:, klo:khi],
            op0=ALU.mult, op1=ALU.add)
        nc.vector.tensor_scalar(
            out=jf[:], in0=jf[:], scalar1=-0.5, scalar2=None, op0=ALU.add)
        jfu = small.tile([P, nsl], U32, name=f"jfu{klo}", tag="cw_jfu")
        nc.vector.tensor_copy(out=jfu[:], in_=jf[:])
        for k in range(klo, khi):
            xneg = xneg_p.tile([P, d], BF16, name="xneg")
            nc.gpsimd.indirect_dma_start(
                out=xneg[:], out_offset=None,
                in_=io["xfb"][:, :],
                in_offset=IndirectOffsetOnAxis(
                    ap=jfu[:, k - klo:k - klo + 1], axis=0),
                bounds_check=n - 1, oob_is_err=False,
            )
            dtile = diff_p.tile([P, d], BF16, name="dtile")
            nc.vector.tensor_tensor(
                out=dtile[:], in0=xrb_sb[:, k * d:(k + 1) * d],
                in1=xneg[:], op=ALU.subtract)
            nc.scalar.activation(
                out=diff_p.tile([P, d], BF16, name="dabs"), in_=dtile[:],
                func=ACTF.Abs, accum_out=san[:, k:k + 1])

    # ---------------- main sweep: strips descending ----------------
    for s in range(ns - 1, -1, -1):
        nb = s // 2               # transposing slots / window fill count
        nk = s // 2 + 1           # sim tasks this strip
        xm_sb = xmp.tile([P, kb * JS], mybir.dt.float8e4, name="xm_sb")
        nc.sync.dma_start(
            out=xm_sb[:].rearrange("p (a b) -> p a b", a=kb),
            in_=io["xm"][:, s * JS:(s + 1) * JS].rearrange(
                "(a p) b -> p a b", p=P),
        )
        xm3 = xm_sb[:].rearrange("p (a b) -> p a b", a=kb)
        if s == 12:
            # xrb load off the critical front (needed only for the tail)
            nc.sync.dma_start(
                out=xrb_sb[:].rearrange("p (a dd) -> p a dd", a=S),
                in_=io["xrb"][:, :].rearrange("(a p) dd -> p a dd", p=P),
            )

        win = winp.tile([P, 4 * 7 * P], BF16, name="win") \
            if nb > 0 else None

        for k in range(nk):
            ps_s = psum.tile([P, JS], F32, name="ps_s", tag="ps")
            matmul_task(k, s, ps_s)
            sstrip = evac.tile([P, JS], BF16, name="sstrip")
            nc.scalar.copy(out=sstrip[:], in_=ps_s[:])
            if s in (2 * k, 2 * k + 1):
                ms = evac.tile([P, JS], BF16, name="msstrip")
                nc.vector.tensor_tensor(
                    out=ms[:], in0=sstrip[:],
                    in1=maskp_sb[:, (s - 2 * k) * JS:(s - 2 * k + 1) * JS],
                    op=ALU.add)
                sstrip = ms
            # row-side chunk max + stage strip to DRAM
            nc.vector.tensor_reduce(
                out=RC[:, k * ns + s:k * ns + s + 1],
                in_=sstrip[:].rearrange("p (o f) -> p o f", o=1),
                axis=AXX, op=ALU.max)
            nc.sync.dma_start(
                out=rowstrips[k * P * ns:(k + 1) * P * ns, :].rearrange(
                    "(p c) f -> p c f", p=P)[:, s:s + 1, :],
                in_=sstrip[:].rearrange("p (o f) -> p o f", o=1))
            # transposes for col side
            if k < nb:
                ptile = pt_pool.tile([P, 4 * P], F32, name="ptile", tag="pt")
                for blk in range(4):
                    nc.tensor.matmul(
                        out=ptile[:, blk * P:(blk + 1) * P],
                        lhsT=sstrip[:, blk * P:(blk + 1) * P],
                        rhs=eye_sb[:], start=True, stop=True,
                    )
                nc.scalar.copy(
                    out=win[:, :4 * nb * P].rearrange(
                        "p (blk sl q) -> p blk sl q", blk=4, sl=nb)[
                            :, :, k:k + 1, :],
                    in_=ptile[:].rearrange(
                        "p (blk o q) -> p blk o q", blk=4, o=1),
                )
        # col-side extraction for this strip's 4 column blocks
        if nb > 0:
            for blk in range(4):
                b = 4 * s + blk
                colm = colm_hi if b >= 32 else colm_lo
                colp = colp_hi if b >= 32 else colp_lo
                i = (b % 8) * 4 + ((b - 32 if b >= 32 else b) // 8)
                wv = win[:, blk * nb * P:(blk + 1) * nb * P]
                nc.vector.tensor_reduce(
                    out=colm[:, i:i + 1],
                    in_=wv.rearrange("p (o f) -> p o f", o=1),
                    axis=AXX, op=ALU.max)
                cmb = small.tile([P, 8], BF16, name="cmb", tag="cmb")
                nc.vector.tensor_copy(
                    out=cmb[:], in_=colm[:, i:i + 1].to_broadcast([P, 8]))
                cp8 = small.tile([P, 8], U32, name="cp8", tag="cp8")
                nc.vector.max_index(out=cp8[:], in_max=cmb[:], in_values=wv)
                nc.vector.tensor_copy(out=colp[:, i:i + 1], in_=cp8[:, 0:1])

        if s == 8:
            # first half (b 32..63, slots 4..7 everywhere) complete
            pack_half(colm_hi, colp_hi, 4, bounce_hi, 4)
            nc.gpsimd.collective_compute(
                "ReduceScatter", ALU.max,
                replica_groups=[list(range(cfg.cores))],
                ins=[bounce_hi.opt()], outs=[bounce_hi_out.opt()],
            )
            for k in range(4, 8):
                row_extract(k)

    # ---------------- second half tail ----------------
    for k in range(4):
        row_extract(k)
    pack_half(colm_lo, colp_lo, 0, bounce_lo, 4)
    nc.gpsimd.collective_compute(
        "ReduceScatter", ALU.max,
        replica_groups=[list(range(cfg.cores))],
        ins=[bounce_lo.opt()], outs=[bounce_lo_out.opt()],
    )
    # RS#1 completed during strips 7..0; mine slots 4-7 while RS#2 runs
    finish_slots(4, 8, bounce_hi_out)

    # d_ap matmuls (PE work overlapping the RS + mining tail)
    for k in range(S):
        for cch in range(cfg.ch):
            ps_y = psum.tile([P, cfg.cw], F32, name="ps_y", tag="ps")
            nc.tensor.matmul(
                out=ps_y[:], lhsT=m2b_sb[:],
                rhs=xrb_sb[:, k * d + cch * cfg.cw:k * d + (cch + 1) * cfg.cw],
                start=True, stop=True,
            )
            y_sc = yabs.tile([P, cfg.cw], F32, name="y_sc")
            nc.scalar.activation(
                out=y_sc[:], in_=ps_y[:], func=ACTF.Abs,
                accum_out=sap[:, k * cfg.ch + cch:k * cfg.ch + cch + 1],
            )

    finish_slots(0, 4, bounce_lo_out)

    # ---------------- final per-row loss ----------------
    fin = pool("fin", 1)
    sap8 = fin.tile([P, S], F32, name="sap8")
    nc.vector.tensor_reduce(
        out=sap8[:], in_=sap[:].rearrange("p (a b) -> p a b", a=S),
        axis=AXX, op=ALU.add)
    t1 = fin.tile([P, S], F32, name="t1")
    nc.vector.tensor_scalar(
        out=t1[:], in0=san[:], scalar1=1.0 / d, scalar2=EPS,
        op0=ALU.mult, op1=ALU.add)
    rec = fin.tile([P, S], F32, name="rec")
    nc.vector.reciprocal(out=rec[:], in_=t1[:])
    t2 = fin.tile([P, S], F32, name="t2")
    nc.vector.tensor_tensor(out=t2[:], in0=sap8[:], in1=rec[:], op=ALU.mult)
    lossv = fin.tile([P, S], F32, name="lossv")
    nc.vector.tensor_scalar(
        out=lossv[:], in0=t2[:], scalar1=0.5 * WEIGHT / d, scalar2=None,
        op0=ALU.mult)
    nc.sync.dma_start(out=io["loss_part"][:, :], in_=lossv[:])

    for p in reversed(list(pools.values())):
        p.release()


def build(cfg: Cfg) -> bass.Bass:
    nc = bacc.Bacc("TRN2", target_bir_lowering=False, debug=False,
                   num_devices=cfg.cores)
    io = {
        "xm": nc.dram_tensor("xm", [cfg.d, cfg.n], mybir.dt.float8e4,
                             kind="ExternalInput").ap(),
        "xs": nc.dram_tensor("xs", [cfg.d, cfg.r], mybir.dt.float8e4,
                             kind="ExternalInput").ap(),
        "xrb": nc.dram_tensor("xrb", [cfg.r, cfg.d], BF16,
                              kind="ExternalInput").ap(),
        "xfb": nc.dram_tensor("xfb", [cfg.n, cfg.d], BF16,
                              kind="ExternalInput").ap(),
        "eye": nc.dram_tensor("eye", [P, P], BF16, kind="ExternalInput").ap(),
        "m2b": nc.dram_tensor("m2b", [P, P], BF16, kind="ExternalInput").ap(),
        "maskp": nc.dram_tensor("maskp", [P, 2 * JS], BF16,
                                kind="ExternalInput").ap(),
        "iota16": nc.dram_tensor("iota16", [P, 16], F32,
                                 kind="ExternalInput").ap(),
        "bmi16": nc.dram_tensor("bmi16", [P, 16], F32,
                                kind="ExternalInput").ap(),
        "p16": nc.dram_tensor("p16", [P, 1], F32, kind="ExternalInput").ap(),
        "c128": nc.dram_tensor("c128", [P, 1], F32,
                               kind="ExternalInput").ap(),
        "negfill": nc.dram_tensor("negfill", [P, 32], F32,
                                  kind="ExternalInput").ap(),
        "loss_part": nc.dram_tensor("loss_part", [P, cfg.slots], F32,
                                    kind="ExternalOutput").ap(),
    }
    with tile.TileContext(nc) as tc:
        _body(tc, cfg, io)
    nc.compile()
    return nc


def make_in_maps(cfg: Cfg, x: np.ndarray) -> list[dict]:
    x = np.ascontiguousarray(x, dtype=np.float32)
    xt_q = np.ascontiguousarray(x.T.astype(ml_dtypes.float8_e4m3))
    x_bf = x.astype(ml_dtypes.bfloat16)

    m2 = np.eye(P, dtype=np.float32)
    for cc in range(P // CHUNK):
        m2[cc * CHUNK:(cc + 1) * CHUNK, cc * CHUNK:(cc + 1) * CHUNK] -= 1.0 / CHUNK
    m2b = m2.astype(ml_dtypes.bfloat16)
    eye = np.eye(P, dtype=np.float32).astype(ml_dtypes.bfloat16)

    iota16 = np.broadcast_to(np.arange(16, dtype=np.float32), (P, 16)).copy()
    bmi16 = (65536.0 - iota16).astype(np.float32)
    p16 = (np.arange(P, dtype=np.float32) * 16).reshape(P, 1)
    negfill = np.full((P, 32), PACK_NEG, dtype=np.float32)

    in_maps = []
    for c in range(cfg.cores):
        rows = np.concatenate(
            [np.arange(1024 * k + 128 * c, 1024 * k + 128 * c + P)
             for k in range(cfg.slots)])
        maskp = np.zeros((P, 2 * JS), dtype=np.float32)
        soff = (c // 4) * JS + (c % 4) * P
        for q in range(P):
            g0 = 4 * (q // 4)
            maskp[q, soff + g0:soff + g0 + 4] = NEG_BIG
        in_maps.append({
            "xm": xt_q,
            "xs": np.ascontiguousarray(xt_q[:, rows]),
            "xrb": np.ascontiguousarray(x_bf[rows]),
            "xfb": x_bf,
            "eye": eye,
            "m2b": m2b,
            "maskp": maskp.astype(ml_dtypes.bfloat16),
            "iota16": iota16,
            "bmi16": bmi16,
            "p16": p16,
            "c128": np.full((P, 1), 128.0 * c, dtype=np.float32),
            "negfill": negfill,
        })
    return in_maps


def reduce_outputs(cfg: Cfg, results: list[dict]) -> np.ndarray:
    total = 0.0
    for res in results:
        total += float(res["loss_part"].astype(np.float64).sum())
    return np.float32(total)


def run(cfg: Cfg, x: np.ndarray, trace: bool = False):
    nc = build(cfg)
    in_maps = make_in_maps(cfg, x)
    out = run_bass_kernel_spmd(nc, in_maps, list(range(cfg.cores)), trace=trace)
    return out


def kernel(x: np.ndarray) -> np.ndarray:
    cfg = Cfg(n=8192, d=2048, cores=8)
    last_err = None
    for _ in range(3):
        try:
            out = run(cfg, x)
            return reduce_outputs(cfg, out.results)
        except Exception as e:  # transient device errors: rebuild + retry
            last_err = e
    raise last_err


# revision 11
# speedup vs baseline: 1.0246x; 1.0246x over previous
"""Trainium2 Bass kernel: contrastive loss with negative mining.

Math:
    centers  = mean over contiguous chunks of 8 rows               [n/8, d]
    x_pos    = x + 0.5*(center - x)        => |x - x_pos| = 0.5*|x - center|
    sim      = x @ x.T                                             [n, n]
    neg_idx  = argmax_j sim[i, j] excluding j in i's group-of-4
    d_an     = mean_d |x - x_neg|,  d_ap = mean_d |x - x_pos|
    loss     = sum( (1/8) * d_ap / (d_an + 1e-7) )

Distribution: sim is symmetric, so only the upper block-triangle is
computed (plus a uniform sliver of duplicates).  Skew decomposition keeps
the program identical on all 8 cores (SPMD): core c owns block-rows
a = 8k + c (k = slot 0..7); slot k is matmul'd against column strips
[2k, 15].  Every sim element (r, j) is then covered either by r's own
row side (j-block >= 8k) or by the transpose of block (j-block, r-block)
computed on j-block's owner.

Per (slot, strip) task: fp8 DoubleRow matmuls -> PSUM f32 -> bf16 SBUF
strip.  Row side: per-strip reduce-max into RC, strip data staged to DRAM;
after a slot's sweep the winning strip is re-fetched by indirect DMA and
FIND_INDEX8 recovers the column.  Col side: PE transposes (strips >=
2k+2) land in per-strip windows; reduce-max + FIND_INDEX8 give per-column
(value, source-row) candidates.  Candidates are packed into f32 as
floor(4*v)*8192 + j (integer-exact, argmax-compatible; quantization
mispicks are benign) and ReduceScatter(max) routes each row's best
candidate to its owner.  Two RS rounds (strips 15..8, then 7..0) overlap
the first half's mining tail with the second half's matmuls.  The owner
unpacks j = w mod 8192, gathers x_neg, and computes exact d_an/d_ap/loss.
"""

import math

import ml_dtypes
import numpy as np

import concourse.bass as bass
import concourse.mybir as mybir
import concourse.tile as tile
from concourse import bacc
from concourse.bass import IndirectOffsetOnAxis
from concourse.bass_utils import run_bass_kernel_spmd

BF16 = mybir.dt.bfloat16
F32 = mybir.dt.float32
U32 = mybir.dt.uint32
ALU = mybir.AluOpType
ACTF = mybir.ActivationFunctionType
AXX = mybir.AxisListType.X

P = 128          # partitions / block height
JS = 512         # strip width (4 blocks)
CHUNK = 8
GROUP = 4
WEIGHT = 1.0 / 8
EPS = 1e-7
NEG_BIG = -1e30          # mask addend / empty-candidate fill
PACK_NEG = -3.0e34       # pre-packed equivalent of NEG_BIG (loses to all real)
RK = 12582912.0          # 1.5*2^23: adding+subtracting rounds f32 to integer


class Cfg:
    def __init__(self, n=8192, d=2048, cores=8, fp8=True):
        self.n, self.d, self.cores, self.fp8 = n, d, cores, fp8
        self.r = n // cores           # rows per core (1024)
        self.slots = self.r // P      # stationary slots per core (8)
        self.ns = n // JS             # column strips (16)
        self.kb = d // P              # contraction blocks (16)
        self.cw = min(d, JS)          # d-chunk width for d_ap matmul
        self.ch = d // self.cw
        assert n == 8192 and d == 2048 and cores == 8


def _body(tc: tile.TileContext, cfg: Cfg, io: dict):
    nc = tc.nc
    pools = {}

    def pool(name, bufs, space="SBUF"):
        if name not in pools:
            pools[name] = tc.alloc_tile_pool(name=name, bufs=bufs, space=space)
        return pools[name]

    n, d, ns, kb = cfg.n, cfg.d, cfg.ns, cfg.kb
    S = cfg.slots   # 8

    # ---------------- resident inputs ----------------
    xs_sb = pool("xs", 1).tile([P, kb * cfg.r], mybir.dt.float8e4, name="xs_sb")
    for k in range(0, kb, 2):
        ke = min(k + 2, kb)
        nc.sync.dma_start(
            out=xs_sb[:, k * cfg.r:ke * cfg.r].rearrange(
                "p (a r) -> p a r", a=ke - k),
            in_=io["xs"][k * P:ke * P, :].rearrange("(a p) r -> p a r", p=P),
        )
    xs3 = xs_sb[:].rearrange("p (a r) -> p a r", a=kb)

    consts = pool("consts", 1)
    eye_sb = consts.tile_from(io["eye"])         # [128,128] bf16 identity
    m2b_sb = consts.tile_from(io["m2b"])         # [128,128] bf16 d_ap matrix
    maskp_sb = consts.tile_from(io["maskp"])     # [128,1024] bf16 diag mask
    iota16_sb = consts.tile_from(io["iota16"])   # [128,16] f32 0..15
    bmi16_sb = consts.tile_from(io["bmi16"])     # [128,16] f32 BIG - iota
    p16_sb = consts.tile_from(io["p16"])         # [128,1] f32 p*16
    c128_sb = consts.tile_from(io["c128"])       # [128,1] f32 128*c
    negfill_sb = consts.tile_from(io["negfill"])  # [128,32] f32 PACK_NEG

    # xrb (bf16 rows of this core) loads late-ish; needed for d_an/d_ap only
    xrb_sb = pool("xrb", 1).tile([P, S * d], BF16, name="xrb_sb")

    # ---------------- DRAM scratch ----------------
    dram = pool("dram", 1, space="DRAM")
    rowstrips = dram.tile([S * P * ns, JS], BF16, name="rowstrips")
    bounce = dram.tile([1, 8192], F32, name="bounce")
    bounce_out = dram.tile([1, 1024], F32, name="bounce_out")

    # ---------------- persistent SBUF state ----------------
    state = pool("state", 1)
    RC = state.tile([P, S * ns], F32, name="RC")          # row chunk maxes
    nc.gpsimd.memset(RC[:], NEG_BIG)
    colm = state.tile([P, 64], F32, name="colm")    # col max per block
    colp = state.tile([P, 64], F32, name="colp")    # col window pos (f32)
    nc.gpsimd.memset(colm[:], NEG_BIG)
    nc.gpsimd.memset(colp[:], 0.0)
    rowm = state.tile([P, S], F32, name="rowm")           # row max per slot
    jrow = state.tile([P, S], F32, name="jrow")           # row argmax col
    wfin = state.tile([P, S], F32, name="wfin")           # final packed
    san = state.tile([P, S], F32, name="san")             # sum |x - xneg|
    sap = state.tile([P, S * cfg.ch], F32, name="sap")    # sum |y| per chunk

    xmp = pool("xm", 2)
    psum = pool("ps", 6, space="PSUM")
    pt_pool = pool("pt", 2, space="PSUM")
    evac = pool("evac", 3)
    winp = pool("win", 2)
    small = pool("small", 2)
    gath = pool("gath", 2)
    xneg_p = pool("xneg", 2)
    diff_p = pool("diff", 2)
    yabs = pool("yabs", 2)

    def matmul_task(k, s, out_ps):
        """fp8 DoubleRow sim matmuls: slot k stationary x strip s."""
        for q in range(0, kb, 2):
            nc.tensor.matmul(
                out=out_ps[:],
                lhsT=xs3[:, q:q + 2, k * P:(k + 1) * P],
                rhs=xm3[:, q:q + 2, :],
                start=(q == 0), stop=(q == kb - 2),
                perf_mode=mybir.MatmulPerfMode.DoubleRow,
            )

    def row_extract(k):
        """After slot k's strips all reduced: find its row-side winner."""
        sm = small.tile([P, 16], F32, name=f"sm{k}", tag="sm")
        # max over strips
        nc.vector.tensor_reduce(
            out=rowm[:, k:k + 1], in_=RC[:, k * ns:(k + 1) * ns].rearrange(
                "p (o f) -> p o f", o=1), axis=AXX, op=ALU.max)
        # strip argmax (lowest strip on ties)
        oh = small.tile([P, 16], F32, name=f"oh{k}", tag="oh")
        nc.vector.tensor_tensor(
            out=oh[:], in0=RC[:, k * ns:(k + 1) * ns],
            in1=rowm[:, k:k + 1].to_broadcast([P, 16]), op=ALU.is_ge)
        nc.vector.tensor_tensor(
            out=sm[:], in0=oh[:], in1=bmi16_sb[:], op=ALU.mult)
        sel = small.tile([P, 1], F32, name=f"sel{k}", tag="sel")
        # sel = BIG - max(oh*(BIG-iota)) = min strip index achieving max
        nc.vector.tensor_reduce(
            out=sel[:], in_=sm[:].rearrange("p (o f) -> p o f", o=1),
            axis=AXX, op=ALU.max)
        nc.vector.tensor_scalar(
            out=sel[:], in0=sel[:], scalar1=-1.0, scalar2=65536.0,
            op0=ALU.mult, op1=ALU.add)
        # gather winning strip from DRAM: row id = p*16 + sel
        offs = small.tile([P, 1], F32, name=f"offs{k}", tag="offs")
        nc.vector.scalar_tensor_tensor(
            out=offs[:], in0=sel[:], scalar=float(k * P * ns), in1=p16_sb[:],
            op0=ALU.add, op1=ALU.add)
        offu = small.tile([P, 1], U32, name=f"offu{k}", tag="offu")
        nc.vector.tensor_copy(out=offu[:], in_=offs[:])
        gstrip = gath.tile([P, JS], BF16, name="gstrip")
        nc.gpsimd.indirect_dma_start(
            out=gstrip[:], out_offset=None,
            in_=rowstrips[:, :],
            in_offset=IndirectOffsetOnAxis(ap=offu[:], axis=0),
            bounds_check=S * P * ns - 1, oob_is_err=False,
        )
        mkb = small.tile([P, 8], BF16, name=f"mkb{k}", tag="mkb")
        nc.vector.tensor_copy(out=mkb[:], in_=rowm[:, k:k + 1].to_broadcast([P, 8]))
        pos8 = small.tile([P, 8], U32, name=f"pos8{k}", tag="pos8")
        nc.vector.max_index(out=pos8[:], in_max=mkb[:], in_values=gstrip[:])
        posf = small.tile([P, 1], F32, name=f"posf{k}", tag="posf")
        nc.vector.tensor_copy(out=posf[:], in_=pos8[:, 0:1])
        # j = sel*512 + pos
        nc.vector.scalar_tensor_tensor(
            out=jrow[:, k:k + 1], in0=sel[:], scalar=float(JS), in1=posf[:],
            op0=ALU.mult, op1=ALU.add)

    def pack_all():
        """Pack col (and row) candidates to w = floor(4v)*8192 + j; write col
        packed to the bounce buffer."""
        # col side: j = 8*p - 7*(p mod 128) + 128*c, all via the
        # RK round-trick (DVE ISA has no mod).  kf = floor(p/128) exactly:
        # round((p+0.5)/128 - 0.5 + RK) - RK.
        kf = small.tile([P, 64], F32, name="kf", tag="pk_kf")
        nc.vector.tensor_scalar(
            out=kf[:], in0=colp[:], scalar1=0.5, scalar2=1.0 / 128,
            op0=ALU.add, op1=ALU.mult)
        nc.vector.tensor_scalar(
            out=kf[:], in0=kf[:], scalar1=-0.5, scalar2=RK,
            op0=ALU.add, op1=ALU.add)
        nc.vector.tensor_scalar(
            out=kf[:], in0=kf[:], scalar1=RK, scalar2=None, op0=ALU.subtract)
        pm = small.tile([P, 64], F32, name="pm", tag="pk_pm")
        nc.vector.scalar_tensor_tensor(
            out=pm[:], in0=kf[:], scalar=-128.0, in1=colp[:],
            op0=ALU.mult, op1=ALU.add)
        t2 = small.tile([P, 64], F32, name="t2", tag="pk_t2")
        nc.vector.tensor_scalar(
            out=t2[:], in0=pm[:], scalar1=-7.0, scalar2=None, op0=ALU.mult)
        jc = small.tile([P, 64], F32, name="jc", tag="pk_jc")
        nc.vector.scalar_tensor_tensor(
            out=jc[:], in0=colp[:], scalar=8.0, in1=t2[:],
            op0=ALU.mult, op1=ALU.add)
        nc.vector.tensor_tensor(
            out=jc[:], in0=jc[:], in1=c128_sb[:].to_broadcast([P, 64]),
            op=ALU.add)
        # value quantized to 0.5: rq = round(2v); w = rq*8192 + j + 0.5
        rq = small.tile([P, 64], F32, name="rq", tag="pk_rq")
        nc.vector.tensor_scalar(
            out=rq[:], in0=colm[:], scalar1=2.0, scalar2=RK,
            op0=ALU.mult, op1=ALU.add)
        nc.vector.tensor_scalar(
            out=rq[:], in0=rq[:], scalar1=RK, scalar2=None, op0=ALU.subtract)
        wc = small.tile([P, 64], F32, name="wc", tag="pk_wc")
        nc.vector.scalar_tensor_tensor(
            out=wc[:], in0=rq[:], scalar=8192.0, in1=jc[:],
            op0=ALU.mult, op1=ALU.add)
        nc.vector.tensor_scalar(
            out=wc[:], in0=wc[:], scalar1=0.5, scalar2=None, op0=ALU.add)
        # scatter to bounce: element (p, i) -> slot cb*1024... within-half:
        # half-local b index i (0..31), global kb half handled by caller.
        # bounce flat index = cb*512 + kbl*128 + p, where i = kbl*8 + ... no:
        # i = b - b0 with b natural order => i = kbl*... b = b0 + i,
        # cb = b mod 8, kbl = (b - b0)//8 = i//8, and i mod 8 = cb' pattern:
        # b natural ascending => i = kbl*8 + cb? b = b0 + kbl*8 + cb yes.
        nc.sync.dma_start(
            out=bounce[:].rearrange("o (cb kb q) -> (o q) cb kb",
                                    cb=8, kb=8),
            in_=wc[:].rearrange("p (cb kb) -> p cb kb", cb=8),
        )

    def finish_slots(klo, khi):
        """RS result -> final winner -> gather x_neg -> d_an for slots
        [klo, khi)."""
        nsl = khi - klo
        colw = small.tile([P, nsl], F32, name=f"colw{klo}", tag="cw_colw")
        nc.sync.dma_start(
            out=colw[:],
            in_=bounce_out[:, klo * P:khi * P].rearrange(
                "o (kk q) -> (o q) kk", q=P),
        )
        # pack row-side candidates for these slots (same format)
        rqr = small.tile([P, nsl], F32, name=f"rqr{klo}", tag="cw_rqr")
        nc.vector.tensor_scalar(
            out=rqr[:], in0=rowm[:, klo:khi], scalar1=2.0, scalar2=RK,
            op0=ALU.mult, op1=ALU.add)
        nc.vector.tensor_scalar(
            out=rqr[:], in0=rqr[:], scalar1=RK, scalar2=None,
            op0=ALU.subtract)
        wr = small.tile([P, nsl], F32, name=f"wr{klo}", tag="cw_wr")
        nc.vector.scalar_tensor_tensor(
            out=wr[:], in0=rqr[:], scalar=8192.0, in1=jrow[:, klo:khi],
            op0=ALU.mult, op1=ALU.add)
        nc.vector.tensor_scalar(
            out=wr[:], in0=wr[:], scalar1=0.5, scalar2=None, op0=ALU.add)
        nc.vector.tensor_tensor(
            out=wfin[:, klo:khi], in0=wr[:], in1=colw[:], op=ALU.max)
        # unpack j = w mod 8192: m = floor(w/8192) via round(w/8192 - 0.5),
        # exact since frac(w/8192) = (j+0.5)/8192 is never 0 or 1/2... then
        # j = w - 8192*m - 0.5.
        mf = small.tile([P, nsl], F32, name=f"mf{klo}", tag="cw_mf")
        nc.vector.tensor_scalar(
            out=mf[:], in0=wfin[:, klo:khi], scalar1=1.0 / 8192,
            scalar2=-0.5, op0=ALU.mult, op1=ALU.add)
        nc.vector.tensor_scalar(
            out=mf[:], in0=mf[:], scalar1=RK, scalar2=None, op0=ALU.add)
        nc.vector.tensor_scalar(
            out=mf[:], in0=mf[:], scalar1=RK, scalar2=None, op0=ALU.subtract)
        jf = small.tile([P, nsl], F32, name=f"jf{klo}", tag="cw_jf")
        nc.vector.scalar_tensor_tensor(
            out=jf[:], in0=mf[:], scalar=-8192.0, in1=wfin[:, klo:khi],
            op0=ALU.mult, op1=ALU.add)
        nc.vector.tensor_scalar(
            out=jf[:], in0=jf[:], scalar1=-0.5, scalar2=None, op0=ALU.add)
        jfu = small.tile([P, nsl], U32, name=f"jfu{klo}", tag="cw_jfu")
        nc.vector.tensor_copy(out=jfu[:], in_=jf[:])
        for k in range(klo, khi):
            xneg = xneg_p.tile([P, d], BF16, name="xneg")
            nc.gpsimd.indirect_dma_start(
                out=xneg[:], out_offset=None,
                in_=io["xfb"][:, :],
                in_offset=IndirectOffsetOnAxis(
                    ap=jfu[:, k - klo:k - klo + 1], axis=0),
                bounds_check=n - 1, oob_is_err=False,
            )
            dtile = diff_p.tile([P, d], BF16, name="dtile")
            nc.vector.tensor_tensor(
                out=dtile[:], in0=xrb_sb[:, k * d:(k + 1) * d],
                in1=xneg[:], op=ALU.subtract)
            nc.scalar.activation(
                out=diff_p.tile([P, d], BF16, name="dabs"), in_=dtile[:],
                func=ACTF.Abs, accum_out=san[:, k:k + 1])

    # ---------------- main sweep: strips descending ----------------
    for s in range(ns - 1, -1, -1):
        nb = s // 2               # transposing slots / window fill count
        nk = s // 2 + 1           # sim tasks this strip
        xm_sb = xmp.tile([P, kb * JS], mybir.dt.float8e4, name="xm_sb")
        nc.sync.dma_start(
            out=xm_sb[:].rearrange("p (a b) -> p a b", a=kb),
            in_=io["xm"][:, s * JS:(s + 1) * JS].rearrange(
                "(a p) b -> p a b", p=P),
        )
        xm3 = xm_sb[:].rearrange("p (a b) -> p a b", a=kb)
        if s == 12:
            # xrb load off the critical front (needed only for the tail)
            nc.sync.dma_start(
                out=xrb_sb[:].rearrange("p (a dd) -> p a dd", a=S),
                in_=io["xrb"][:, :].rearrange("(a p) dd -> p a dd", p=P),
            )

        win = winp.tile([P, 4 * 7 * P], BF16, name="win") \
            if nb > 0 else None

        for k in range(nk):
            ps_s = psum.tile([P, JS], F32, name="ps_s", tag="ps")
            matmul_task(k, s, ps_s)
            sstrip = evac.tile([P, JS], BF16, name="sstrip")
            nc.scalar.copy(out=sstrip[:], in_=ps_s[:])
            if s in (2 * k, 2 * k + 1):
                ms = evac.tile([P, JS], BF16, name="msstrip")
                nc.vector.tensor_tensor(
                    out=ms[:], in0=sstrip[:],
                    in1=maskp_sb[:, (s - 2 * k) * JS:(s - 2 * k + 1) * JS],
                    op=ALU.add)
                sstrip = ms
            # row-side chunk max + stage strip to DRAM
            nc.vector.tensor_reduce(
                out=RC[:, k * ns + s:k * ns + s + 1],
                in_=sstrip[:].rearrange("p (o f) -> p o f", o=1),
                axis=AXX, op=ALU.max)
            nc.sync.dma_start(
                out=rowstrips[k * P * ns:(k + 1) * P * ns, :].rearrange(
                    "(p c) f -> p c f", p=P)[:, s:s + 1, :],
                in_=sstrip[:].rearrange("p (o f) -> p o f", o=1))
            # transposes for col side
            if k < nb:
                ptile = pt_pool.tile([P, 4 * P], F32, name="ptile", tag="pt")
                for blk in range(4):
                    nc.tensor.matmul(
                        out=ptile[:, blk * P:(blk + 1) * P],
                        lhsT=sstrip[:, blk * P:(blk + 1) * P],
                        rhs=eye_sb[:], start=True, stop=True,
                    )
                nc.scalar.copy(
                    out=win[:, :4 * nb * P].rearrange(
                        "p (blk sl q) -> p blk sl q", blk=4, sl=nb)[
                            :, :, k:k + 1, :],
                    in_=ptile[:].rearrange(
                        "p (blk o q) -> p blk o q", blk=4, o=1),
                )
        # col-side extraction for this strip's 4 column blocks
        if nb > 0:
            for blk in range(4):
                b = 4 * s + blk
                i = (b % 8) * 8 + b // 8
                wv = win[:, blk * nb * P:(blk + 1) * nb * P]
                nc.vector.tensor_reduce(
                    out=colm[:, i:i + 1],
                    in_=wv.rearrange("p (o f) -> p o f", o=1),
                    axis=AXX, op=ALU.max)
                cmb = small.tile([P, 8], BF16, name="cmb", tag="cmb")
                nc.vector.tensor_copy(
                    out=cmb[:], in_=colm[:, i:i + 1].to_broadcast([P, 8]))
                cp8 = small.tile([P, 8], U32, name="cp8", tag="cp8")
                nc.vector.max_index(out=cp8[:], in_max=cmb[:], in_values=wv)
                nc.vector.tensor_copy(out=colp[:, i:i + 1], in_=cp8[:, 0:1])

        if s == 8:
            # slots 4..7 row side complete; extract while strips 7..0 run
            for k in range(4, 8):
                row_extract(k)

    # ---------------- tail: one collective after all PE work ----------------
    for k in range(4):
        row_extract(k)
    pack_all()

    # d_ap matmuls emitted before the collective (it is a global barrier)
    for k in range(S):
        for cch in range(cfg.ch):
            ps_y = psum.tile([P, cfg.cw], F32, name="ps_y", tag="ps")
            nc.tensor.matmul(
                out=ps_y[:], lhsT=m2b_sb[:],
                rhs=xrb_sb[:, k * d + cch * cfg.cw:k * d + (cch + 1) * cfg.cw],
                start=True, stop=True,
            )
            y_sc = yabs.tile([P, cfg.cw], F32, name="y_sc")
            nc.scalar.activation(
                out=y_sc[:], in_=ps_y[:], func=ACTF.Abs,
                accum_out=sap[:, k * cfg.ch + cch:k * cfg.ch + cch + 1],
            )

    nc.gpsimd.collective_compute(
        "ReduceScatter", ALU.max,
        replica_groups=[list(range(cfg.cores))],
        ins=[bounce.opt()], outs=[bounce_out.opt()],
    )
    finish_slots(0, 8)

    # ---------------- final per-row loss ----------------
    fin = pool("fin", 1)
    sap8 = fin.tile([P, S], F32, name="sap8")
    nc.vector.tensor_reduce(
        out=sap8[:], in_=sap[:].rearrange("p (a b) -> p a b", a=S),
        axis=AXX, op=ALU.add)
    t1 = fin.tile([P, S], F32, name="t1")
    nc.vector.tensor_scalar(
        out=t1[:], in0=san[:], scalar1=1.0 / d, scalar2=EPS,
        op0=ALU.mult, op1=ALU.add)
    rec = fin.tile([P, S], F32, name="rec")
    nc.vector.reciprocal(out=rec[:], in_=t1[:])
    t2 = fin.tile([P, S], F32, name="t2")
    nc.vector.tensor_tensor(out=t2[:], in0=sap8[:], in1=rec[:], op=ALU.mult)
    lossv = fin.tile([P, S], F32, name="lossv")
    nc.vector.tensor_scalar(
        out=lossv[:], in0=t2[:], scalar1=0.5 * WEIGHT / d, scalar2=None,
        op0=ALU.mult)
    nc.sync.dma_start(out=io["loss_part"][:, :], in_=lossv[:])

    for p in reversed(list(pools.values())):
        p.release()


def build(cfg: Cfg) -> bass.Bass:
    nc = bacc.Bacc("TRN2", target_bir_lowering=False, debug=False,
                   num_devices=cfg.cores)
    io = {
        "xm": nc.dram_tensor("xm", [cfg.d, cfg.n], mybir.dt.float8e4,
                             kind="ExternalInput").ap(),
        "xs": nc.dram_tensor("xs", [cfg.d, cfg.r], mybir.dt.float8e4,
                             kind="ExternalInput").ap(),
        "xrb": nc.dram_tensor("xrb", [cfg.r, cfg.d], BF16,
                              kind="ExternalInput").ap(),
        "xfb": nc.dram_tensor("xfb", [cfg.n, cfg.d], BF16,
                              kind="ExternalInput").ap(),
        "eye": nc.dram_tensor("eye", [P, P], BF16, kind="ExternalInput").ap(),
        "m2b": nc.dram_tensor("m2b", [P, P], BF16, kind="ExternalInput").ap(),
        "maskp": nc.dram_tensor("maskp", [P, 2 * JS], BF16,
                                kind="ExternalInput").ap(),
        "iota16": nc.dram_tensor("iota16", [P, 16], F32,
                                 kind="ExternalInput").ap(),
        "bmi16": nc.dram_tensor("bmi16", [P, 16], F32,
                                kind="ExternalInput").ap(),
        "p16": nc.dram_tensor("p16", [P, 1], F32, kind="ExternalInput").ap(),
        "c128": nc.dram_tensor("c128", [P, 1], F32,
                               kind="ExternalInput").ap(),
        "negfill": nc.dram_tensor("negfill", [P, 32], F32,
                                  kind="ExternalInput").ap(),
        "loss_part": nc.dram_tensor("loss_part", [P, cfg.slots], F32,
                                    kind="ExternalOutput").ap(),
    }
    with tile.TileContext(nc) as tc:
        _body(tc, cfg, io)
    nc.compile()
    return nc


def make_in_maps(cfg: Cfg, x: np.ndarray) -> list[dict]:
    x = np.ascontiguousarray(x, dtype=np.float32)
    xt_q = np.ascontiguousarray(x.T.astype(ml_dtypes.float8_e4m3))
    x_bf = x.astype(ml_dtypes.bfloat16)

    m2 = np.eye(P, dtype=np.float32)
    for cc in range(P // CHUNK):
        m2[cc * CHUNK:(cc + 1) * CHUNK, cc * CHUNK:(cc + 1) * CHUNK] -= 1.0 / CHUNK
    m2b = m2.astype(ml_dtypes.bfloat16)
    eye = np.eye(P, dtype=np.float32).astype(ml_dtypes.bfloat16)

    iota16 = np.broadcast_to(np.arange(16, dtype=np.float32), (P, 16)).copy()
    bmi16 = (65536.0 - iota16).astype(np.float32)
    p16 = (np.arange(P, dtype=np.float32) * 16).reshape(P, 1)
    negfill = np.full((P, 32), PACK_NEG, dtype=np.float32)

    in_maps = []
    for c in range(cfg.cores):
        rows = np.concatenate(
            [np.arange(1024 * k + 128 * c, 1024 * k + 128 * c + P)
             for k in range(cfg.slots)])
        maskp = np.zeros((P, 2 * JS), dtype=np.float32)
        soff = (c // 4) * JS + (c % 4) * P
        for q in range(P):
            g0 = 4 * (q // 4)
            maskp[q, soff + g0:soff + g0 + 4] = NEG_BIG
        in_maps.append({
            "xm": xt_q,
            "xs": np.ascontiguousarray(xt_q[:, rows]),
            "xrb": np.ascontiguousarray(x_bf[rows]),
            "xfb": x_bf,
            "eye": eye,
            "m2b": m2b,
            "maskp": maskp.astype(ml_dtypes.bfloat16),
            "iota16": iota16,
            "bmi16": bmi16,
            "p16": p16,
            "c128": np.full((P, 1), 128.0 * c, dtype=np.float32),
            "negfill": negfill,
        })
    return in_maps


def reduce_outputs(cfg: Cfg, results: list[dict]) -> np.ndarray:
    total = 0.0
    for res in results:
        total += float(res["loss_part"].astype(np.float64).sum())
    return np.float32(total)


def run(cfg: Cfg, x: np.ndarray, trace: bool = False):
    nc = build(cfg)
    in_maps = make_in_maps(cfg, x)
    out = run_bass_kernel_spmd(nc, in_maps, list(range(cfg.cores)), trace=trace)
    return out


def kernel(x: np.ndarray) -> np.ndarray:
    cfg = Cfg(n=8192, d=2048, cores=8)
    last_err = None
    for _ in range(3):
        try:
            out = run(cfg, x)
            return reduce_outputs(cfg, out.results)
        except Exception as e:  # transient device errors: rebuild + retry
            last_err = e
    raise last_err


# revision 12
# speedup vs baseline: 1.2900x; 1.2591x over previous
"""Trainium2 Bass kernel: contrastive loss with negative mining.

Math:
    centers  = mean over contiguous chunks of 8 rows               [n/8, d]
    x_pos    = x + 0.5*(center - x)        => |x - x_pos| = 0.5*|x - center|
    sim      = x @ x.T                                             [n, n]
    neg_idx  = argmax_j sim[i, j] excluding j in i's group-of-4
    d_ap     = mean_d |x - x_pos|,  d_an = mean_d |x - x_neg|
    loss     = sum( (1/8) * d_ap / (d_an + 1e-7) )

Distribution: data-parallel over rows, 8 NeuronCores, 1024 rows each.
Every core receives the full x.T (fp8) plus a bf16 copy of x in its own
DRAM, so no collectives are needed; per-row losses are returned and summed
on host.

Per core:
  - sim rows are fp8e4m3 DoubleRow matmuls (stationary = xT slice of this
    core's rows, moving = full xT) in 512-wide column strips, f32 PSUM
    accumulation over 8 k-pair blocks, evacuated to SBUF as bf16 (ScalarE).
  - Per strip, DVE max/max_index extract the top-8 values + indices per
    row.  A row's excluded group-of-4 spans at most 4 of its strip's
    top-8, so the best valid candidate always survives.
  - i-tiles are processed in two passes of 4 (the moving operand is read
    twice) so the first pass's negative-mining tail (candidate combine,
    x_neg gather, d_an) overlaps the second pass's matmuls.
  - Candidate combine is batched over 4 i-tiles: global indices,
    group-exclusion masking via compares against per-partition group
    bounds (input data), then argmax value + min-index-of-max reductions.
  - x_neg rows are gathered (bf16) from DRAM with a GPSIMD indirect DMA;
    d_an is a bf16 DVE subtract + ScalarE Abs+accumulate (f32 accum).
  - d_ap uses y = (I - blockdiag(ones(8,8)/8)) @ x_tile (bf16 matmuls,
    emitted last so they overlap the final tail) with ScalarE
    Abs+accumulate.
"""

import math

import ml_dtypes
import numpy as np

import concourse.bass as bass
import concourse.mybir as mybir
import concourse.tile as tile
from concourse import bacc
from concourse.bass import IndirectOffsetOnAxis
from concourse.bass_utils import run_bass_kernel_spmd

BF16 = mybir.dt.bfloat16
F32 = mybir.dt.float32
U32 = mybir.dt.uint32
ALU = mybir.AluOpType
ACTF = mybir.ActivationFunctionType
AXX = mybir.AxisListType.X

P = 128         # partitions / row-tile height
JS = 512        # similarity column-strip width
CHUNK = 8       # rows averaged per center
GROUP = 4       # negative-mining exclusion window
WEIGHT = 1.0 / 8
EPS = 1e-7
NEG_BIG = -1e30
BIGI = 65536.0  # index bias for the min-index-of-max trick


class Cfg:
    def __init__(self, n=8192, d=2048, cores=8, fp8=True):
        self.n, self.d, self.cores, self.fp8 = n, d, cores, fp8
        self.r = n // cores            # rows per core
        self.it = self.r // P          # i-tiles per core
        self.nj = n // JS              # column strips
        self.kb = d // P               # contraction blocks
        self.cw = min(d, JS)           # d-chunk width for the d_ap matmul
        self.ch = d // self.cw         # number of d-chunks
        self.nq = self.nj * 8          # candidates per i-tile
        self.gi = min(4, self.it)      # i-tiles per pass / combine batch
        assert n % (cores * P) == 0 and d % P == 0 and n % JS == 0
        assert d % self.cw == 0 and self.it % self.gi == 0


def _body(tc: tile.TileContext, cfg: Cfg, io: dict):
    nc = tc.nc
    ctxpools = {}

    def pool(name, bufs, space="SBUF"):
        if name not in ctxpools:
            ctxpools[name] = tc.alloc_tile_pool(name=name, bufs=bufs, space=space)
        return ctxpools[name]

    sim_dt = mybir.dt.float8e4 if cfg.fp8 else BF16

    # resident stationary xT slice: [128, KB*R], k-block major.
    # Chunked DMAs so the first matmuls can start before the full load lands.
    xs_sb = pool("xs", 1).tile([P, cfg.kb * cfg.r], sim_dt, name="xs_sb")
    for k in range(0, cfg.kb, 2):
        ke = min(k + 2, cfg.kb)
        nc.sync.dma_start(
            out=xs_sb[:, k * cfg.r:ke * cfg.r].rearrange(
                "p (a r) -> p a r", a=ke - k),
            in_=io["xs"][k * P:ke * P, :].rearrange("(a p) r -> p a r", p=P),
        )

    # resident bf16 x rows: d_an minuend + d_ap input.  Loaded after the
    # second moving strip so the 4MB transfer stays off the critical front.
    xrb_sb = pool("xrb", 1).tile([P, cfg.it * cfg.d], BF16, name="xrb_sb")

    psum = pool("ps", 8, space="PSUM")
    small = pool("small", 1)
    sap = small.tile([P, cfg.it * cfg.ch], F32, name="sap")    # sum|y| per chunk
    san = small.tile([P, cfg.it], F32, name="san")             # sum|x-xneg|
    idxall = small.tile([P, cfg.it], U32, name="idxall")       # neg indices
    # per-pass candidate tiles (separate so pass 0's combine does not
    # falsely depend on pass 1's writes)
    npass = cfg.it // cfg.gi
    cv_sb = [small.tile([P, cfg.gi * cfg.nq], BF16, name=f"cv{g}", tag=f"cv{g}")
             for g in range(npass)]
    ci_sb = [small.tile([P, cfg.gi * cfg.nq], U32, name=f"ci{g}", tag=f"ci{g}")
             for g in range(npass)]

    consts = pool("consts", 1)
    m2b_sb = consts.tile_from(io["m2b"])                     # [128,128] bf16
    offs_sb = consts.tile_from(io["offsw"])                  # [128,IT*NQ] f32
    g0_sb = consts.tile_from(io["g0w"])                      # [128,IT*NQ] f32
    g3_sb = consts.tile_from(io["g3w"])                      # [128,IT*NQ] f32

    xmp = pool("xm", 2)
    evac = pool("evac", 4)
    comb = pool("comb", 1)
    xneg_p = pool("xneg", 2)
    diff_p = pool("diff", 2)
    dabs_p = pool("dabs", 2)

    xs3 = xs_sb[:].rearrange("p (a r) -> p a r", a=cfg.kb)
    G = cfg.gi
    W = G * cfg.nq

    for a in range(0, cfg.it, G):
        # ---- sim strips for i-tiles [a, a+G) + per-strip top-8 ----
        for j in range(cfg.nj):
            xm_sb = xmp.tile([P, cfg.kb * JS], sim_dt, name="xm_sb")
            nc.sync.dma_start(
                out=xm_sb[:].rearrange("p (a b) -> p a b", a=cfg.kb),
                in_=io["xm"][:, j * JS:(j + 1) * JS].rearrange(
                    "(a p) b -> p a b", p=P),
            )
            xm3 = xm_sb[:].rearrange("p (a b) -> p a b", a=cfg.kb)
            if a == 0 and j == 1:
                nc.sync.dma_start(
                    out=xrb_sb[:].rearrange("p (a d) -> p a d", a=cfg.it),
                    in_=io["xrb"][:, :].rearrange("(a p) d -> p a d", p=P),
                )
            for it in range(a, a + G):
                ps_s = psum.tile([P, JS], F32, name="ps_s", tag="ps")
                if cfg.fp8:
                    for k in range(0, cfg.kb, 2):
                        nc.tensor.matmul(
                            out=ps_s[:],
                            lhsT=xs3[:, k:k + 2, it * P:(it + 1) * P],
                            rhs=xm3[:, k:k + 2, :],
                            start=(k == 0), stop=(k == cfg.kb - 2),
                            perf_mode=mybir.MatmulPerfMode.DoubleRow,
                        )
                else:
                    for k in range(cfg.kb):
                        nc.tensor.matmul(
                            out=ps_s[:],
                            lhsT=xs_sb[:, k * cfg.r + it * P:
                                       k * cfg.r + (it + 1) * P],
                            rhs=xm_sb[:, k * JS:(k + 1) * JS],
                            start=(k == 0), stop=(k == cfg.kb - 1),
                        )
                sstrip = evac.tile([P, JS], BF16, name="sstrip")
                nc.scalar.copy(out=sstrip[:], in_=ps_s[:])
                q0 = ((it - a) * cfg.nj + j) * 8
                nc.vector.max(out=cv_sb[a // G][:, q0:q0 + 8], in_=sstrip[:])
                nc.vector.max_index(
                    out=ci_sb[a // G][:, q0:q0 + 8],
                    in_max=cv_sb[a // G][:, q0:q0 + 8],
                    in_values=sstrip[:],
                )

        # ---- batched candidate combine for this pass's i-tiles ----
        lo, hi = a * cfg.nq, (a + G) * cfg.nq
        cif = comb.tile([P, W], F32, name="cif")
        nc.vector.tensor_copy(out=cif[:], in_=ci_sb[a // G][:])
        gidx = comb.tile([P, W], F32, name="gidx")
        nc.vector.tensor_tensor(
            out=gidx[:], in0=cif[:], in1=offs_sb[:, lo:hi], op=ALU.add)
        b1 = comb.tile([P, W], F32, name="b1")
        nc.vector.tensor_tensor(
            out=b1[:], in0=gidx[:], in1=g0_sb[:, lo:hi], op=ALU.is_ge)
        b2 = comb.tile([P, W], F32, name="b2")
        nc.vector.tensor_tensor(
            out=b2[:], in0=gidx[:], in1=g3_sb[:, lo:hi], op=ALU.is_le)
        msk = comb.tile([P, W], F32, name="msk")
        nc.vector.scalar_tensor_tensor(
            out=msk[:], in0=b1[:], scalar=NEG_BIG, in1=b2[:],
            op0=ALU.mult, op1=ALU.mult)
        cvf = comb.tile([P, W], F32, name="cvf")
        nc.vector.tensor_copy(out=cvf[:], in_=cv_sb[a // G][:])
        mv = comb.tile([P, W], F32, name="mv")
        nc.vector.tensor_tensor(out=mv[:], in0=cvf[:], in1=msk[:], op=ALU.add)
        mv3 = mv[:].rearrange("p (g q) -> p g q", g=G)
        mxg = comb.tile([P, G], F32, name="mxg")
        nc.vector.tensor_reduce(out=mxg[:], in_=mv3, axis=AXX, op=ALU.max)
        sel = comb.tile([P, W], F32, name="sel")
        nc.vector.tensor_tensor(
            out=sel[:].rearrange("p (g q) -> p g q", g=G), in0=mv3,
            in1=mxg[:].to_broadcast([P, G, cfg.nq]), op=ALU.is_ge)
        pick = comb.tile([P, W], F32, name="pick")
        nc.vector.scalar_tensor_tensor(
            out=pick[:], in0=gidx[:], scalar=BIGI, in1=sel[:],
            op0=ALU.subtract, op1=ALU.mult)
        mng = comb.tile([P, G], F32, name="mng")
        nc.vector.tensor_reduce(
            out=mng[:], in_=pick[:].rearrange("p (g q) -> p g q", g=G),
            axis=AXX, op=ALU.min)
        idxg = comb.tile([P, G], F32, name="idxg")
        nc.vector.tensor_scalar(
            out=idxg[:], in0=mng[:], scalar1=BIGI, scalar2=None, op0=ALU.add)
        nc.vector.tensor_copy(out=idxall[:, a:a + G], in_=idxg[:])

        # ---- gather x_neg (bf16) + d_an for this pass's i-tiles ----
        for it in range(a, a + G):
            xneg = xneg_p.tile([P, cfg.d], BF16, name="xneg")
            nc.gpsimd.indirect_dma_start(
                out=xneg[:], out_offset=None,
                in_=io["xfb"][:, :],
                in_offset=IndirectOffsetOnAxis(ap=idxall[:, it:it + 1], axis=0),
                # an OOB index must not fault the device; skip it instead
                bounds_check=cfg.n - 1, oob_is_err=False,
            )
            diff = diff_p.tile([P, cfg.d], BF16, name="diff")
            nc.vector.tensor_tensor(
                out=diff[:], in0=xrb_sb[:, it * cfg.d:(it + 1) * cfg.d],
                in1=xneg[:], op=ALU.subtract,
            )
            dabs = dabs_p.tile([P, cfg.d], BF16, name="dabs")
            nc.scalar.activation(
                out=dabs[:], in_=diff[:], func=ACTF.Abs,
                accum_out=san[:, it:it + 1],
            )

    # ---- d_ap (emitted last; PE work overlaps the final pass's tail):
    #      y = M2 @ x_tile, sum_d |y|  (bf16 matmuls) ----
    yabs = pool("yabs", 2)
    for it in range(cfg.it):
        for c in range(cfg.ch):
            ps_y = psum.tile([P, cfg.cw], F32, name="ps_y", tag="ps")
            nc.tensor.matmul(
                out=ps_y[:], lhsT=m2b_sb[:],
                rhs=xrb_sb[:, it * cfg.d + c * cfg.cw:
                           it * cfg.d + (c + 1) * cfg.cw],
                start=True, stop=True,
            )
            y_sc = yabs.tile([P, cfg.cw], F32, name="y_sc")
            nc.scalar.activation(
                out=y_sc[:], in_=ps_y[:], func=ACTF.Abs,
                accum_out=sap[:, it * cfg.ch + c: it * cfg.ch + c + 1],
            )

    # ---- Final: per-row loss ----
    fin = pool("fin", 1)
    sap8 = fin.tile([P, cfg.it], F32, name="sap8")
    sap3 = sap[:].rearrange("p (a b) -> p a b", a=cfg.it)
    nc.vector.tensor_reduce(out=sap8[:], in_=sap3, axis=AXX, op=ALU.add)
    t1 = fin.tile([P, cfg.it], F32, name="t1")
    nc.vector.tensor_scalar(
        out=t1[:], in0=san[:], scalar1=1.0 / cfg.d, scalar2=EPS,
        op0=ALU.mult, op1=ALU.add,
    )
    rec = fin.tile([P, cfg.it], F32, name="rec")
    nc.vector.reciprocal(out=rec[:], in_=t1[:])
    t2 = fin.tile([P, cfg.it], F32, name="t2")
    nc.vector.tensor_tensor(out=t2[:], in0=sap8[:], in1=rec[:], op=ALU.mult)
    lossv = fin.tile([P, cfg.it], F32, name="lossv")
    nc.vector.tensor_scalar(
        out=lossv[:], in0=t2[:], scalar1=0.5 * WEIGHT / cfg.d, scalar2=None,
        op0=ALU.mult,
    )
    nc.sync.dma_start(out=io["loss_part"][:, :], in_=lossv[:])
    nc.sync.dma_start(out=io["nidx"][:, :], in_=idxall[:])

    for p in reversed(list(ctxpools.values())):
        p.release()


def build(cfg: Cfg) -> bass.Bass:
    nc = bacc.Bacc("TRN2", target_bir_lowering=False, debug=False)
    sim_dt = mybir.dt.float8e4 if cfg.fp8 else BF16
    nqw = cfg.it * cfg.nq
    io = {
        "xm": nc.dram_tensor("xm", [cfg.d, cfg.n], sim_dt, kind="ExternalInput").ap(),
        "xs": nc.dram_tensor("xs", [cfg.d, cfg.r], sim_dt, kind="ExternalInput").ap(),
        "xrb": nc.dram_tensor("xrb", [cfg.r, cfg.d], BF16, kind="ExternalInput").ap(),
        "xfb": nc.dram_tensor("xfb", [cfg.n, cfg.d], BF16, kind="ExternalInput").ap(),
        "m2b": nc.dram_tensor("m2b", [P, P], BF16, kind="ExternalInput").ap(),
        "offsw": nc.dram_tensor("offsw", [P, nqw], F32, kind="ExternalInput").ap(),
        "g0w": nc.dram_tensor("g0w", [P, nqw], F32, kind="ExternalInput").ap(),
        "g3w": nc.dram_tensor("g3w", [P, nqw], F32, kind="ExternalInput").ap(),
        "loss_part": nc.dram_tensor("loss_part", [P, cfg.it], F32, kind="ExternalOutput").ap(),
        "nidx": nc.dram_tensor("nidx", [P, cfg.it], U32, kind="ExternalOutput").ap(),
    }
    with tile.TileContext(nc) as tc:
        _body(tc, cfg, io)
    nc.compile()
    return nc


def make_in_maps(cfg: Cfg, x: np.ndarray) -> list[dict]:
    x = np.ascontiguousarray(x, dtype=np.float32)
    sim_np = ml_dtypes.float8_e4m3 if cfg.fp8 else ml_dtypes.bfloat16
    xt_q = np.ascontiguousarray(x.T.astype(sim_np))
    x_bf = x.astype(ml_dtypes.bfloat16)

    m2 = np.eye(P, dtype=np.float32)
    for c in range(P // CHUNK):
        m2[c * CHUNK:(c + 1) * CHUNK, c * CHUNK:(c + 1) * CHUNK] -= 1.0 / CHUNK
    m2b = m2.astype(ml_dtypes.bfloat16)

    # per-candidate global column offset (same pattern for every i-tile block)
    offs1 = np.zeros(cfg.nq, dtype=np.float32)
    for j in range(cfg.nj):
        offs1[j * 8:(j + 1) * 8] = j * JS
    offsw = np.broadcast_to(
        np.tile(offs1, cfg.it), (P, cfg.it * cfg.nq)).copy()

    pvec = np.arange(P, dtype=np.float32)
    in_maps = []
    for c in range(cfg.cores):
        g0w = np.zeros((P, cfg.it * cfg.nq), dtype=np.float32)
        for it in range(cfg.it):
            col = c * cfg.r + it * P + (pvec // GROUP) * GROUP
            g0w[:, it * cfg.nq:(it + 1) * cfg.nq] = col[:, None]
        in_maps.append({
            "xm": xt_q,
            "xs": np.ascontiguousarray(xt_q[:, c * cfg.r:(c + 1) * cfg.r]),
            "xrb": np.ascontiguousarray(x_bf[c * cfg.r:(c + 1) * cfg.r]),
            "xfb": x_bf,
            "m2b": m2b,
            "offsw": offsw,
            "g0w": g0w,
            "g3w": g0w + (GROUP - 1),
        })
    return in_maps


def reduce_outputs(cfg: Cfg, results: list[dict]) -> np.ndarray:
    total = 0.0
    for res in results:
        total += float(res["loss_part"].astype(np.float64).sum())
    return np.float32(total)


def run(cfg: Cfg, x: np.ndarray, trace: bool = False):
    nc = build(cfg)
    in_maps = make_in_maps(cfg, x)
    out = run_bass_kernel_spmd(nc, in_maps, list(range(cfg.cores)), trace=trace)
    return out


def kernel(x: np.ndarray) -> np.ndarray:
    cfg = Cfg(n=8192, d=2048, cores=8)
    last_err = None
    for _ in range(3):
        try:
            out = run(cfg, x)
            return reduce_outputs(cfg, out.results)
        except Exception as e:  # transient device errors: rebuild + retry
            last_err = e
    raise last_err



# revision 14
# speedup vs baseline: 1.2974x; 1.0057x over previous
"""Trainium2 Bass kernel: contrastive loss with negative mining.

Math:
    centers  = mean over contiguous chunks of 8 rows               [n/8, d]
    x_pos    = x + 0.5*(center - x)        => |x - x_pos| = 0.5*|x - center|
    sim      = x @ x.T                                             [n, n]
    neg_idx  = argmax_j sim[i, j] excluding j in i's group-of-4
    d_ap     = mean_d |x - x_pos|,  d_an = mean_d |x - x_neg|
    loss     = sum( (1/8) * d_ap / (d_an + 1e-7) )

Distribution: data-parallel over rows, 8 NeuronCores, 1024 rows each.
Every core receives the full x.T (fp8) plus a bf16 copy of x in its own
DRAM, so no collectives are needed; per-row losses are returned and summed
on host.

Per core:
  - sim rows are fp8e4m3 DoubleRow matmuls (stationary = xT slice of this
    core's rows, moving = full xT) in 512-wide column strips, f32 PSUM
    accumulation over 8 k-pair blocks, evacuated to SBUF as bf16 (ScalarE).
  - Per strip, DVE max/max_index extract the top-8 values + indices per
    row.  A row's excluded group-of-4 spans at most 4 of its strip's
    top-8, so the best valid candidate always survives.
  - i-tiles are processed in two passes of 4 (the moving operand is read
    twice) so the first pass's negative-mining tail (candidate combine,
    x_neg gather, d_an) overlaps the second pass's matmuls.
  - Candidate combine is batched over 4 i-tiles: global indices,
    group-exclusion masking via compares against per-partition group
    bounds (input data), then argmax value + min-index-of-max reductions.
  - x_neg rows are gathered (bf16) from DRAM with a GPSIMD indirect DMA;
    d_an is a bf16 DVE subtract + ScalarE Abs+accumulate (f32 accum).
  - d_ap uses y = (I - blockdiag(ones(8,8)/8)) @ x_tile (bf16 matmuls,
    emitted last so they overlap the final tail) with ScalarE
    Abs+accumulate.
"""

import math

import ml_dtypes
import numpy as np

import concourse.bass as bass
import concourse.mybir as mybir
import concourse.tile as tile
from concourse import bacc
from concourse.bass import IndirectOffsetOnAxis
from concourse.bass_utils import run_bass_kernel_spmd

BF16 = mybir.dt.bfloat16
F32 = mybir.dt.float32
U32 = mybir.dt.uint32
ALU = mybir.AluOpType
ACTF = mybir.ActivationFunctionType
AXX = mybir.AxisListType.X

P = 128         # partitions / row-tile height
JS = 512        # similarity column-strip width
CHUNK = 8       # rows averaged per center
GROUP = 4       # negative-mining exclusion window
WEIGHT = 1.0 / 8
EPS = 1e-7
NEG_BIG = -1e30
BIGI = 65536.0  # index bias for the min-index-of-max trick


class Cfg:
    def __init__(self, n=8192, d=2048, cores=8, fp8=True):
        self.n, self.d, self.cores, self.fp8 = n, d, cores, fp8
        self.r = n // cores            # rows per core
        self.it = self.r // P          # i-tiles per core
        self.nj = n // JS              # column strips
        self.kb = d // P               # contraction blocks
        self.cw = min(d, JS)           # d-chunk width for the d_ap matmul
        self.ch = d // self.cw         # number of d-chunks
        self.nq = self.nj * 8          # candidates per i-tile
        self.gi = min(4, self.it)      # i-tiles per pass / combine batch
        assert n % (cores * P) == 0 and d % P == 0 and n % JS == 0
        assert d % self.cw == 0 and self.it % self.gi == 0


def _body(tc: tile.TileContext, cfg: Cfg, io: dict):
    nc = tc.nc
    ctxpools = {}

    def pool(name, bufs, space="SBUF"):
        if name not in ctxpools:
            ctxpools[name] = tc.alloc_tile_pool(name=name, bufs=bufs, space=space)
        return ctxpools[name]

    sim_dt = mybir.dt.float8e4 if cfg.fp8 else BF16

    # resident stationary xT slice: [128, KB*R], k-block major.
    # Chunked DMAs so the first matmuls can start before the full load lands.
    xs_sb = pool("xs", 1).tile([P, cfg.kb * cfg.r], sim_dt, name="xs_sb")
    for k in range(0, cfg.kb, 2):
        ke = min(k + 2, cfg.kb)
        nc.sync.dma_start(
            out=xs_sb[:, k * cfg.r:ke * cfg.r].rearrange(
                "p (a r) -> p a r", a=ke - k),
            in_=io["xs"][k * P:ke * P, :].rearrange("(a p) r -> p a r", p=P),
        )

    # resident bf16 x rows: d_an minuend + d_ap input.  Loaded after the
    # second moving strip so the 4MB transfer stays off the critical front.
    xrb_sb = pool("xrb", 1).tile([P, cfg.it * cfg.d], BF16, name="xrb_sb")

    psum = pool("ps", 8, space="PSUM")
    small = pool("small", 1)
    sap = small.tile([P, cfg.it * cfg.ch], F32, name="sap")    # sum|y| per chunk
    san = small.tile([P, cfg.it], F32, name="san")             # sum|x-xneg|
    idxall = small.tile([P, cfg.it], U32, name="idxall")       # neg indices
    # per-pass candidate tiles (separate so pass 0's combine does not
    # falsely depend on pass 1's writes)
    npass = cfg.it // cfg.gi
    cv_sb = [small.tile([P, cfg.gi * cfg.nq], BF16, name=f"cv{g}", tag=f"cv{g}")
             for g in range(npass)]
    ci_sb = [small.tile([P, cfg.gi * cfg.nq], U32, name=f"ci{g}", tag=f"ci{g}")
             for g in range(npass)]

    consts = pool("consts", 1)
    late_consts = {}

    xmp = pool("xm", 2)
    evac = pool("evac", 4)
    comb = pool("comb", 1)
    xneg_p = pool("xneg", 2)
    diff_p = pool("diff", 2)
    dabs_p = pool("dabs", 2)

    xs3 = xs_sb[:].rearrange("p (a r) -> p a r", a=cfg.kb)
    G = cfg.gi
    W = G * cfg.nq

    for a in range(0, cfg.it, G):
        # ---- sim strips for i-tiles [a, a+G) + per-strip top-8 ----
        for j in range(cfg.nj):
            xm_sb = xmp.tile([P, cfg.kb * JS], sim_dt, name="xm_sb")
            if a == 0 and j == 0:
                for kc in range(0, cfg.kb, 2):
                    nc.sync.dma_start(
                        out=xm_sb[:, kc * JS:(kc + 2) * JS].rearrange(
                            "p (a b) -> p a b", a=2),
                        in_=io["xm"][kc * P:(kc + 2) * P,
                                     j * JS:(j + 1) * JS].rearrange(
                            "(a p) b -> p a b", p=P),
                    )
            else:
                nc.sync.dma_start(
                    out=xm_sb[:].rearrange("p (a b) -> p a b", a=cfg.kb),
                    in_=io["xm"][:, j * JS:(j + 1) * JS].rearrange(
                        "(a p) b -> p a b", p=P),
                )
            xm3 = xm_sb[:].rearrange("p (a b) -> p a b", a=cfg.kb)
            if a == 0 and j == 1:
                nc.sync.dma_start(
                    out=xrb_sb[:].rearrange("p (a d) -> p a d", a=cfg.it),
                    in_=io["xrb"][:, :].rearrange("(a p) d -> p a d", p=P),
                )
                late_consts["m2b"] = consts.tile_from(io["m2b"], name="m2b_sb")
                late_consts["offs"] = consts.tile_from(io["offsw"], name="offs_sb")
                late_consts["g0"] = consts.tile_from(io["g0w"], name="g0_sb")
                late_consts["g3"] = consts.tile_from(io["g3w"], name="g3_sb")
            for it in range(a, a + G):
                ps_s = psum.tile([P, JS], F32, name="ps_s", tag="ps")
                if cfg.fp8:
                    for k in range(0, cfg.kb, 2):
                        nc.tensor.matmul(
                            out=ps_s[:],
                            lhsT=xs3[:, k:k + 2, it * P:(it + 1) * P],
                            rhs=xm3[:, k:k + 2, :],
                            start=(k == 0), stop=(k == cfg.kb - 2),
                            perf_mode=mybir.MatmulPerfMode.DoubleRow,
                        )
                else:
                    for k in range(cfg.kb):
                        nc.tensor.matmul(
                            out=ps_s[:],
                            lhsT=xs_sb[:, k * cfg.r + it * P:
                                       k * cfg.r + (it + 1) * P],
                            rhs=xm_sb[:, k * JS:(k + 1) * JS],
                            start=(k == 0), stop=(k == cfg.kb - 1),
                        )
                sstrip = evac.tile([P, JS], BF16, name="sstrip")
                nc.scalar.copy(out=sstrip[:], in_=ps_s[:])
                q0 = ((it - a) * cfg.nj + j) * 8
                nc.vector.max(out=cv_sb[a // G][:, q0:q0 + 8], in_=sstrip[:])
                nc.vector.max_index(
                    out=ci_sb[a // G][:, q0:q0 + 8],
                    in_max=cv_sb[a // G][:, q0:q0 + 8],
                    in_values=sstrip[:],
                )

        # ---- batched candidate combine for this pass's i-tiles ----
        lo, hi = a * cfg.nq, (a + G) * cfg.nq
        cif = comb.tile([P, W], F32, name="cif")
        nc.vector.tensor_copy(out=cif[:], in_=ci_sb[a // G][:])
        gidx = comb.tile([P, W], F32, name="gidx")
        nc.vector.tensor_tensor(
            out=gidx[:], in0=cif[:], in1=late_consts["offs"][:, lo:hi], op=ALU.add)
        b1 = comb.tile([P, W], F32, name="b1")
        nc.vector.tensor_tensor(
            out=b1[:], in0=gidx[:], in1=late_consts["g0"][:, lo:hi], op=ALU.is_ge)
        b2 = comb.tile([P, W], F32, name="b2")
        nc.vector.tensor_tensor(
            out=b2[:], in0=gidx[:], in1=late_consts["g3"][:, lo:hi], op=ALU.is_le)
        msk = comb.tile([P, W], F32, name="msk")
        nc.vector.scalar_tensor_tensor(
            out=msk[:], in0=b1[:], scalar=NEG_BIG, in1=b2[:],
            op0=ALU.mult, op1=ALU.mult)
        cvf = comb.tile([P, W], F32, name="cvf")
        nc.vector.tensor_copy(out=cvf[:], in_=cv_sb[a // G][:])
        mv = comb.tile([P, W], F32, name="mv")
        nc.vector.tensor_tensor(out=mv[:], in0=cvf[:], in1=msk[:], op=ALU.add)
        mv3 = mv[:].rearrange("p (g q) -> p g q", g=G)
        mxg = comb.tile([P, G], F32, name="mxg")
        nc.vector.tensor_reduce(out=mxg[:], in_=mv3, axis=AXX, op=ALU.max)
        sel = comb.tile([P, W], F32, name="sel")
        nc.vector.tensor_tensor(
            out=sel[:].rearrange("p (g q) -> p g q", g=G), in0=mv3,
            in1=mxg[:].to_broadcast([P, G, cfg.nq]), op=ALU.is_ge)
        pick = comb.tile([P, W], F32, name="pick")
        nc.vector.scalar_tensor_tensor(
            out=pick[:], in0=gidx[:], scalar=BIGI, in1=sel[:],
            op0=ALU.subtract, op1=ALU.mult)
        mng = comb.tile([P, G], F32, name="mng")
        nc.vector.tensor_reduce(
            out=mng[:], in_=pick[:].rearrange("p (g q) -> p g q", g=G),
            axis=AXX, op=ALU.min)
        idxg = comb.tile([P, G], F32, name="idxg")
        nc.vector.tensor_scalar(
            out=idxg[:], in0=mng[:], scalar1=BIGI, scalar2=None, op0=ALU.add)
        nc.vector.tensor_copy(out=idxall[:, a:a + G], in_=idxg[:])

        # ---- gather x_neg (bf16) + d_an for this pass's i-tiles ----
        for it in range(a, a + G):
            xneg = xneg_p.tile([P, cfg.d], BF16, name="xneg")
            nc.gpsimd.indirect_dma_start(
                out=xneg[:], out_offset=None,
                in_=io["xfb"][:, :],
                in_offset=IndirectOffsetOnAxis(ap=idxall[:, it:it + 1], axis=0),
                # an OOB index must not fault the device; skip it instead
                bounds_check=cfg.n - 1, oob_is_err=False,
            )
            diff = diff_p.tile([P, cfg.d], BF16, name="diff")
            nc.vector.tensor_tensor(
                out=diff[:], in0=xrb_sb[:, it * cfg.d:(it + 1) * cfg.d],
                in1=xneg[:], op=ALU.subtract,
            )
            dabs = dabs_p.tile([P, cfg.d], BF16, name="dabs")
            nc.scalar.activation(
                out=dabs[:], in_=diff[:], func=ACTF.Abs,
                accum_out=san[:, it:it + 1],
            )

    # ---- d_ap (emitted last; PE work overlaps the final pass's tail):
    #      y = M2 @ x_tile, sum_d |y|  (bf16 matmuls) ----
    yabs = pool("yabs", 2)
    for it in range(cfg.it):
        for c in range(cfg.ch):
            ps_y = psum.tile([P, cfg.cw], F32, name="ps_y", tag="ps")
            nc.tensor.matmul(
                out=ps_y[:], lhsT=late_consts["m2b"][:],
                rhs=xrb_sb[:, it * cfg.d + c * cfg.cw:
                           it * cfg.d + (c + 1) * cfg.cw],
                start=True, stop=True,
            )
            y_sc = yabs.tile([P, cfg.cw], F32, name="y_sc")
            nc.scalar.activation(
                out=y_sc[:], in_=ps_y[:], func=ACTF.Abs,
                accum_out=sap[:, it * cfg.ch + c: it * cfg.ch + c + 1],
            )

    # ---- Final: per-row loss ----
    fin = pool("fin", 1)
    sap8 = fin.tile([P, cfg.it], F32, name="sap8")
    sap3 = sap[:].rearrange("p (a b) -> p a b", a=cfg.it)
    nc.vector.tensor_reduce(out=sap8[:], in_=sap3, axis=AXX, op=ALU.add)
    t1 = fin.tile([P, cfg.it], F32, name="t1")
    nc.vector.tensor_scalar(
        out=t1[:], in0=san[:], scalar1=1.0 / cfg.d, scalar2=EPS,
        op0=ALU.mult, op1=ALU.add,
    )
    rec = fin.tile([P, cfg.it], F32, name="rec")
    nc.vector.reciprocal(out=rec[:], in_=t1[:])
    t2 = fin.tile([P, cfg.it], F32, name="t2")
    nc.vector.tensor_tensor(out=t2[:], in0=sap8[:], in1=rec[:], op=ALU.mult)
    lossv = fin.tile([P, cfg.it], F32, name="lossv")
    nc.vector.tensor_scalar(
        out=lossv[:], in0=t2[:], scalar1=0.5 * WEIGHT / cfg.d, scalar2=None,
        op0=ALU.mult,
    )
    nc.sync.dma_start(out=io["loss_part"][:, :], in_=lossv[:])
    nc.sync.dma_start(out=io["nidx"][:, :], in_=idxall[:])

    for p in reversed(list(ctxpools.values())):
        p.release()


def build(cfg: Cfg) -> bass.Bass:
    nc = bacc.Bacc("TRN2", target_bir_lowering=False, debug=False)
    sim_dt = mybir.dt.float8e4 if cfg.fp8 else BF16
    nqw = cfg.it * cfg.nq
    io = {
        "xm": nc.dram_tensor("xm", [cfg.d, cfg.n], sim_dt, kind="ExternalInput").ap(),
        "xs": nc.dram_tensor("xs", [cfg.d, cfg.r], sim_dt, kind="ExternalInput").ap(),
        "xrb": nc.dram_tensor("xrb", [cfg.r, cfg.d], BF16, kind="ExternalInput").ap(),
        "xfb": nc.dram_tensor("xfb", [cfg.n, cfg.d], BF16, kind="ExternalInput").ap(),
        "m2b": nc.dram_tensor("m2b", [P, P], BF16, kind="ExternalInput").ap(),
        "offsw": nc.dram_tensor("offsw", [P, nqw], F32, kind="ExternalInput").ap(),
        "g0w": nc.dram_tensor("g0w", [P, nqw], F32, kind="ExternalInput").ap(),
        "g3w": nc.dram_tensor("g3w", [P, nqw], F32, kind="ExternalInput").ap(),
        "loss_part": nc.dram_tensor("loss_part", [P, cfg.it], F32, kind="ExternalOutput").ap(),
        "nidx": nc.dram_tensor("nidx", [P, cfg.it], U32, kind="ExternalOutput").ap(),
    }
    with tile.TileContext(nc) as tc:
        _body(tc, cfg, io)
    nc.compile()
    return nc


def make_in_maps(cfg: Cfg, x: np.ndarray) -> list[dict]:
    x = np.ascontiguousarray(x, dtype=np.float32)
    sim_np = ml_dtypes.float8_e4m3 if cfg.fp8 else ml_dtypes.bfloat16
    xt_q = np.ascontiguousarray(x.T.astype(sim_np))
    x_bf = x.astype(ml_dtypes.bfloat16)

    m2 = np.eye(P, dtype=np.float32)
    for c in range(P // CHUNK):
        m2[c * CHUNK:(c + 1) * CHUNK, c * CHUNK:(c + 1) * CHUNK] -= 1.0 / CHUNK
    m2b = m2.astype(ml_dtypes.bfloat16)

    # per-candidate global column offset (same pattern for every i-tile block)
    offs1 = np.zeros(cfg.nq, dtype=np.float32)
    for j in range(cfg.nj):
        offs1[j * 8:(j + 1) * 8] = j * JS
    offsw = np.broadcast_to(
        np.tile(offs1, cfg.it), (P, cfg.it * cfg.nq)).copy()

    pvec = np.arange(P, dtype=np.float32)
    in_maps = []
    for c in range(cfg.cores):
        g0w = np.zeros((P, cfg.it * cfg.nq), dtype=np.float32)
        for it in range(cfg.it):
            col = c * cfg.r + it * P + (pvec // GROUP) * GROUP
            g0w[:, it * cfg.nq:(it + 1) * cfg.nq] = col[:, None]
        in_maps.append({
            "xm": xt_q,
            "xs": np.ascontiguousarray(xt_q[:, c * cfg.r:(c + 1) * cfg.r]),
            "xrb": np.ascontiguousarray(x_bf[c * cfg.r:(c + 1) * cfg.r]),
            "xfb": x_bf,
            "m2b": m2b,
            "offsw": offsw,
            "g0w": g0w,
            "g3w": g0w + (GROUP - 1),
        })
    return in_maps


def reduce_outputs(cfg: Cfg, results: list[dict]) -> np.ndarray:
    total = 0.0
    for res in results:
        total += float(res["loss_part"].astype(np.float64).sum())
    return np.float32(total)


def run(cfg: Cfg, x: np.ndarray, trace: bool = False):
    nc = build(cfg)
    in_maps = make_in_maps(cfg, x)
    out = run_bass_kernel_spmd(nc, in_maps, list(range(cfg.cores)), trace=trace)
    return out


def kernel(x: np.ndarray) -> np.ndarray:
    cfg = Cfg(n=8192, d=2048, cores=8)
    last_err = None
    for _ in range(3):
        try:
            out = run(cfg, x)
            return reduce_outputs(cfg, out.results)
        except Exception as e:  # transient device errors: rebuild + retry
            last_err = e
    raise last_err

